# revision 1
# baseline (speedup 1.0000x reference)
"""Trainium2 Bass kernel for nn_AssignAttention (softmax over the query axis).

Math (per batch b):
  q = (query @ Wq)  [N, C] -> heads [N, H, hd]
  k = (key   @ Wk)  [S, C] -> heads [S, H, hd]
  raw[h, n, s] = (q_h @ k_h^T) * hd^-0.5
  attn = softmax(raw, axis=n)                  # normalize over queries, per (h, s)
  attn = attn / max(sum_s attn, 1)             # clamp-normalize over s, per (h, n)
  out[n, h*hd:  ] = sum_s attn[h, n, s] * key[s, h*hd: (h+1)*hd]
  returns (out, out_style) with out_style == out

Distribution: data-parallel over B=16 across 8 NeuronCores (2 batches/core).

Per-core dataflow (all matmuls bf16, accumulation f32):
  - key is cast-DMA'd (f32->bf16) into SBUF in natural [s, c] layout (= V).
  - keyT obtained with PE tile transposes; k-projection computed directly in
    transposed [c_out, s] layout: kT = Wk^T-contraction over c_in on partitions.
  - scores[s-part, n-free] = kT_h.T @ qT_h  (K=hd=64).
  - exp via ScalarE with scale folded in; accum_out gives the softmax
    denominator D[s] (sum over the free axis n) for free.
  - 1/D[s] is folded into V rows (4x fewer elements than scaling attn).
  - second matmul contracts s: out_acc[n, c] += e[s,n].T @ (v[s,c]/D[s]),
    div[n] += e[s,n].T @ (1/D[s]); final scale by 1/max(div,1) per n.
"""

import os
import threading

import numpy as np

STAGES = os.environ.get("K_STAGES", "abcd")
NT_LIM = int(os.environ.get("K_NT", "0"))  # 0 = full
NB_LIM = int(os.environ.get("K_NB", "0"))

B, N, S, C, H = 16, 256, 4096, 512, 8
HD = C // H
NCORES = 8
BL = B // NCORES  # batches per core
SCALE = float(HD) ** -0.5

_cache = {}
_lock = threading.Lock()


def _build():
    from contextlib import ExitStack

    import concourse.bass as bass
    import concourse.tile as tile
    from concourse import bacc, mybir
    from concourse.masks import make_identity

    f32 = mybir.dt.float32
    bf16 = mybir.dt.bfloat16

    nc = bacc.Bacc(
        "TRN2",
        target_bir_lowering=False,
        debug=False,
        enable_asserts=False,
        num_devices=NCORES,
    )
    q_ap = nc.dram_tensor("query", [BL, N, C], f32, kind="ExternalInput").ap()
    k_ap = nc.dram_tensor("key", [BL, S, C], f32, kind="ExternalInput").ap()
    wq_ap = nc.dram_tensor("Wq", [C, C], f32, kind="ExternalInput").ap()
    wk_ap = nc.dram_tensor("Wk", [C, C], f32, kind="ExternalInput").ap()
    out_ap = nc.dram_tensor("out", [BL, N, C], f32, kind="ExternalOutput").ap()
    out2_ap = nc.dram_tensor("out_style", [BL, N, C], f32, kind="ExternalOutput").ap()

    NT = S // 128          # 32 s-tiles of 128
    TL = NT_LIM if NT_LIM else NT  # stage-C tiles (bisect knob)
    NJ = S // 512          # 8 macro chunks of 512 rows
    NCK = C // 128         # 4 c_in chunks
    NM = C // 128          # 4 c_out chunks

    with tile.TileContext(nc) as tc, ExitStack() as ctx:
        const = ctx.enter_context(tc.tile_pool(name="const", bufs=1))
        # weights, bf16, layout [c_in_chunk(part=128), k*C + c_out]
        wq_bf = const.tile([128, NCK * C], bf16)
        wk_bf = const.tile([128, NCK * C], bf16)
        nc.gpsimd.dma_start(
            wq_bf[:].rearrange("p (k c) -> p k c", k=NCK),
            wq_ap.rearrange("(k p) c -> p k c", k=NCK),
        )
        nc.gpsimd.dma_start(
            wk_bf[:].rearrange("p (k c) -> p k c", k=NCK),
            wk_ap.rearrange("(k p) c -> p k c", k=NCK),
        )
        ident = const.tile([128, 128], bf16)
        make_identity(nc, ident[:])

        # SBUF pools
        kb_pool = ctx.enter_context(tc.tile_pool(name="kb", bufs=2))
        ktp_pool = ctx.enter_context(tc.tile_pool(name="ktp", bufs=2))
        ktin_pool = ctx.enter_context(tc.tile_pool(name="ktin", bufs=2))
        qpool = ctx.enter_context(tc.tile_pool(name="qpool", bufs=2))
        epool = ctx.enter_context(tc.tile_pool(name="epool", bufs=3))
        spool = ctx.enter_context(tc.tile_pool(name="spool", bufs=3))
        opool = ctx.enter_context(tc.tile_pool(name="opool", bufs=2))

        # PSUM pools (8 banks total: 2 + 1 + 2*1 + 2 + 1 = 8)
        trp_pool = ctx.enter_context(tc.tile_pool(name="trp", bufs=1, space="PSUM"))
        kprj_pool = ctx.enter_context(tc.tile_pool(name="kprj", bufs=1, space="PSUM"))
        sc_pool = ctx.enter_context(tc.tile_pool(name="sc", bufs=2, space="PSUM"))
        oacc_pool = ctx.enter_context(tc.tile_pool(name="oacc", bufs=1, space="PSUM"))
        dacc_pool = ctx.enter_context(tc.tile_pool(name="dacc", bufs=1, space="PSUM"))

        nbatch = NB_LIM if NB_LIM else BL
        for b in range(nbatch):
            # ---------------- Stage A: q path ----------------
            qf_bf = qpool.tile([128, 2 * C], bf16, tag="qf")
            nc.gpsimd.dma_start(
                qf_bf[:].rearrange("p (j c) -> p j c", j=2),
                q_ap[b].rearrange("(j p) c -> p j c", j=2),
            )
            # transpose query -> qT [c(part, by chunk), n]
            qt_sb = qpool.tile([128, NCK * N], bf16, tag="qt")
            for j in range(2):
                tp = trp_pool.tile([128, 1024], bf16, tag="trp")
                for ck in range(NCK):
                    nc.tensor.transpose(
                        tp[:, ck * 128 : (ck + 1) * 128],
                        qf_bf[:, j * C + ck * 128 : j * C + (ck + 1) * 128],
                        ident[:],
                    )
                for ck in range(NCK):
                    nc.vector.tensor_copy(
                        qt_sb[:, ck * N + j * 128 : ck * N + j * 128 + 128],
                        tp[:, ck * 128 : (ck + 1) * 128],
                    )
            # q projection (transposed out): qTp [c_out(part by chunk m), n]
            qtp = qpool.tile([128, NM * N], bf16, tag="qtp")
            for m in range(NM):
                pq = kprj_pool.tile([128, 512], f32, tag="kprj")
                for k in range(NCK):
                    nc.tensor.matmul(
                        pq[:, :N],
                        lhsT=wq_bf[:, k * C + m * 128 : k * C + (m + 1) * 128],
                        rhs=qt_sb[:, k * N : (k + 1) * N],
                        start=(k == 0),
                        stop=(k == NCK - 1),
                    )
                nc.vector.tensor_copy(qtp[:, m * N : (m + 1) * N], pq[:, :N])

            # ---------------- Stage B: k path ----------------
            kb = kb_pool.tile([128, NT * C], bf16, tag="kb")  # natural [s, c] (= V)
            ktp = ktp_pool.tile([128, NM * S], bf16, tag="ktp")  # kT [c_out, s]
            for j in range(NJ):
                # load 512 rows of key, cast f32->bf16 during DMA
                nc.gpsimd.dma_start(
                    kb[:, 4 * j * C : 4 * (j + 1) * C].rearrange(
                        "p (t c) -> p t c", t=4
                    ),
                    k_ap[b, j * 512 : (j + 1) * 512, :].rearrange(
                        "(t p) c -> p t c", t=4
                    ),
                )
                # transpose to keyT chunks -> ktin[:, ck*512 + tt*128]
                ktin = ktin_pool.tile([128, 2048], bf16, tag="ktin")
                for ckp in range(2):
                    tp = trp_pool.tile([128, 1024], bf16, tag="trp")
                    for tt in range(4):
                        t = 4 * j + tt
                        for cc in range(2):
                            ck = ckp * 2 + cc
                            nc.tensor.transpose(
                                tp[:, cc * 512 + tt * 128 : cc * 512 + tt * 128 + 128],
                                kb[:, t * C + ck * 128 : t * C + (ck + 1) * 128],
                                ident[:],
                            )
                    nc.vector.tensor_copy(
                        ktin[:, ckp * 1024 : (ckp + 1) * 1024], tp[:]
                    )
                # k projection, transposed output [c_out(part), s]
                for m in range(NM):
                    pk = kprj_pool.tile([128, 512], f32, tag="kprj")
                    for k in range(NCK):
                        nc.tensor.matmul(
                            pk[:],
                            lhsT=wk_bf[:, k * C + m * 128 : k * C + (m + 1) * 128],
                            rhs=ktin[:, k * 512 : (k + 1) * 512],
                            start=(k == 0),
                            stop=(k == NCK - 1),
                        )
                    nc.vector.tensor_copy(
                        ktp[:, m * S + j * 512 : m * S + (j + 1) * 512], pk[:]
                    )

            # ---------------- Stage C: attention ----------------
            if "c" not in STAGES:
                # dump something derived from ktp/qtp so nothing is dead
                for ncn in range(2):
                    osb = opool.tile([128, C], f32, tag="osb")
                    nc.vector.tensor_copy(osb[:], ktp[:, ncn * C : (ncn + 1) * C])
                    nc.sync.dma_start(
                        out_ap[b, ncn * 128 : (ncn + 1) * 128, :], osb[:]
                    )
                    nc.sync.dma_start(
                        out2_ap[b, ncn * 128 : (ncn + 1) * 128, :], osb[:]
                    )
                continue
            oacc = oacc_pool.tile([128, 16 * HD], f32, tag="oacc")
            dacc = dacc_pool.tile([128, 16], f32, tag="dacc")
            for t in range(TL):
                # scores for all 8 heads: 4 psum tiles of 2 heads each.
                # Heads sharing a PSUM bank must use the SAME PE row group
                # (base partition) -- mixed row-groups writing one bank is an
                # NRT_EXEC_UNIT_UNRECOVERABLE device crash. Tile h-pairs by
                # equal parity: (0,2), (1,3), (4,6), (5,7).
                sc_heads = [(0, 2), (1, 3), (4, 6), (5, 7)]
                scs = []
                for half in range(4):
                    sc = sc_pool.tile([128, 512], f32, tag="sc")
                    scs.append(sc)
                    for hh, h in enumerate(sc_heads[half]):
                        m, hp = h // 2, (h % 2) * 64
                        nc.tensor.matmul(
                            sc[:, hh * N : (hh + 1) * N],
                            lhsT=ktp[
                                hp : hp + 64, m * S + t * 128 : m * S + t * 128 + 128
                            ],
                            rhs=qtp[hp : hp + 64, m * N : (m + 1) * N],
                            start=True,
                            stop=True,
                        )
                et = epool.tile([128, H * N], bf16, tag="et")
                den = spool.tile([128, H], f32, tag="den")
                if "e" in STAGES:
                    for half in range(4):
                        nc.vector.tensor_copy(
                            et[:, half * 512 : (half + 1) * 512], scs[half][:]
                        )
                    continue
                for h in range(H):
                    sc_idx = (h // 4) * 2 + (h % 2)
                    sc_pos = (h // 2) % 2
                    nc.scalar.activation(
                        et[:, h * N : (h + 1) * N],
                        scs[sc_idx][:, sc_pos * N : (sc_pos + 1) * N],
                        mybir.ActivationFunctionType.Exp,
                        scale=SCALE,
                        accum_out=den[:, h : h + 1],
                    )
                if "s" in STAGES:
                    continue
                rt = spool.tile([128, H], f32, tag="rt")
                nc.vector.reciprocal(rt[:], den[:])
                rbf = spool.tile([128, H], bf16, tag="rbf")
                nc.vector.tensor_copy(rbf[:], rt[:])
                vaug = spool.tile([128, H * HD], bf16, tag="vaug")
                for h in range(H):
                    nc.vector.tensor_scalar_mul(
                        vaug[:, h * HD : (h + 1) * HD],
                        kb[:, t * C + h * HD : t * C + (h + 1) * HD],
                        rt[:, h : h + 1],
                    )
                if "v" in STAGES:
                    continue
                # one accumulation "group" per PSUM bank: start only on the
                # first matmul into the bank (t==0), stop on the last (t==31).
                # t==0/t==31 wrapped in a critical section to pin PE order.
                import contextlib

                crit = (
                    tc.tile_critical()
                    if (t == 0 or t == TL - 1) and "x" not in STAGES
                    else contextlib.nullcontext()
                )
                with crit:
                    for h in range(H):
                        for ncn in range(2):
                            g = h * 2 + ncn
                            lhsT = et[:, h * N + ncn * 128 : h * N + ncn * 128 + 128]
                            nc.tensor.matmul(
                                oacc[:, g * HD : (g + 1) * HD],
                                lhsT=lhsT,
                                rhs=vaug[:, h * HD : (h + 1) * HD],
                                start=(t == 0 and g in (0, 8)),
                                stop=(t == TL - 1 and g in (7, 15)),
                                skip_group_check=True,
                            )
                            nc.tensor.matmul(
                                dacc[:, g : g + 1],
                                lhsT=lhsT,
                                rhs=rbf[:, h : h + 1],
                                start=(t == 0 and g == 0),
                                stop=(t == TL - 1 and g == 15),
                                skip_group_check=True,
                            )

            # ---------------- Stage D: epilogue ----------------
            if "s" in STAGES or "v" in STAGES:
                for ncn in range(2):
                    osb = opool.tile([128, C], f32, tag="osb")
                    nc.vector.tensor_copy(osb[:], et[:, ncn * C : (ncn + 1) * C])
                    nc.sync.dma_start(out_ap[b, ncn * 128 : (ncn + 1) * 128, :], osb[:])
                    nc.sync.dma_start(out2_ap[b, ncn * 128 : (ncn + 1) * 128, :], osb[:])
                continue
            dm = spool.tile([128, 16], f32, tag="dm")
            nc.vector.tensor_scalar_max(dm[:], dacc[:], 1.0)
            rdiv = spool.tile([128, 16], f32, tag="rdiv")
            nc.vector.reciprocal(rdiv[:], dm[:])
            for ncn in range(2):
                osb = opool.tile([128, C], f32, tag="osb")
                for h in range(H):
                    g = h * 2 + ncn
                    nc.vector.tensor_scalar_mul(
                        osb[:, h * HD : (h + 1) * HD],
                        oacc[:, g * HD : (g + 1) * HD],
                        rdiv[:, g : g + 1],
                    )
                nc.sync.dma_start(out_ap[b, ncn * 128 : (ncn + 1) * 128, :], osb[:])
                nc.sync.dma_start(out2_ap[b, ncn * 128 : (ncn + 1) * 128, :], osb[:])

    nc.compile()
    return nc


def _get_nc():
    with _lock:
        if "nc" not in _cache:
            _cache["nc"] = _build()
        return _cache["nc"]


def kernel(query, key, Wq, Wk):
    from concourse.bass_utils import run_bass_kernel_spmd

    nc = _get_nc()
    query = np.ascontiguousarray(query, dtype=np.float32)
    key = np.ascontiguousarray(key, dtype=np.float32)
    Wq = np.ascontiguousarray(Wq, dtype=np.float32)
    Wk = np.ascontiguousarray(Wk, dtype=np.float32)
    in_maps = [
        {
            "query": query[c * BL : (c + 1) * BL],
            "key": key[c * BL : (c + 1) * BL],
            "Wq": Wq,
            "Wk": Wk,
        }
        for c in range(NCORES)
    ]
    res = run_bass_kernel_spmd(nc, in_maps, core_ids=list(range(NCORES)))
    out = np.concatenate([r["out"] for r in res.results], axis=0)
    out_style = np.concatenate([r["out_style"] for r in res.results], axis=0)
    return out, out_style

